# revision 52
# baseline (speedup 1.0000x reference)
"""Trainium2 (8 NeuronCores) kernel for a GPT-2 style causal attention block.

Reference math (per batch b):
    qkv = x @ W_attn + b_attn            # [T, 3E]
    q,k,v split -> heads H=16, D=64
    att = softmax(mask(q k^T / sqrt(D))) # causal mask
    y   = (att @ v) @ W_proj + b_proj    # [T, E]

Sharding (8 cores, no collectives):
    core c = (batch b = c//2, head-group hg = c%2 of 8 heads).
    Each core computes a PARTIAL y[b] = O_local @ W_proj[rows of its heads].
    Host sums the two partials per batch and adds b_proj (exact, commutes).

Device kernel per core (all bf16 matmuls, fp32 PSUM accumulate):
    phase 1: Q^T, K^T (feats on partitions) and V (rows on partitions) via
             matmuls from host-fed x^T and W shards.  1/sqrt(D) is folded
             into the Q columns of W on the host (exact: /8 is a pow2).
    phase 2: per (head-pair, q-chunk of 512): S^T tiles [128 k, 512 q] on
             PE, exp on ACT (no max-subtraction needed: scores are O(1) by
             construction).  The two heads' K=64 S^T matmuls are packed into
             one PSUM tile [kpos, j, q] and emitted back-to-back: they land
             on different PE row halves (tile_position auto-derived from the
             kT/qT base partition) and different PSUM banks, so each k-tile's
             S pair runs CONCURRENTLY on the array (~2x on the S stream).
             Causal structure: k-tiles above the diagonal are skipped,
             matmuls/exp on diagonal tiles are trimmed to live columns, and
             the 128-wide mixed band is masked by multiplying with a 128x128
             triangular tile.  O'^T accumulates with a V' that has a
             ones-column appended -> row 64 of O' is the softmax
             denominator.  Normalization happens off-PSUM: one [65,512] copy,
             DRAM-bounce broadcast of the denominator row, fast reciprocal,
             multiply into O^T.
    phase 3: y_partial = O @ W_proj_shard, PSUM -> SBUF -> DRAM (bf16;
             the host sums the two partials per batch in f32).

Schedule (the real perf lever on HW): the Tile scheduler is a per-engine
priority heap (priority = emission order) gated by readiness.  The
attention phases are ACT(exp)-paced, and any PE micro-idle risks the HAM
clock gate re-throttling the PE to 1.2 GHz (the original schedule lost
~45us to one 75us cold stretch).  So the dense matmul work (V, QK, proj)
is staggered across the attention chunks as filler: minimal prologue
(qk0 rc0 first -- its inputs land first -- then V rt0-3), V tail + qk0
tail + qk1 into attn(0), qk2 into attn(1), qk3 (reversed rc to match
attn(3)'s descending qc order) into attn(2), proj per-chunk into
attn(3).  Early-phase PSUM->SBUF copies run on the (exp-idle) scalar
engine so the DVE has no copy backlog that would throttle the psA slot
turnaround when attention starts.

Measured on HW and rejected (CoreSim passes, silicon disagrees): K=1 PE
broadcast matmuls (garbage), gpsimd partition_broadcast (NaN), a merged
4-bank exp tile (PSUM single-buffering stalls), late qk3 emission (NaN),
finer input-DMA slicing (DGE trigger-bound).

Also measured and rejected in a later session (each vs the 279us baseline):
  - PE pre-warm dummy matmuls during the DMA ramp: lifts the HAM 1.2 GHz
    cold throttle ~5us earlier, but the denser early profile (x8 cores)
    trips the chip-level P0 power limiter and the whole body then runs at
    2.0 instead of 2.4 GHz (dense MM spacing 259ns vs 216ns): 285 -> 332us,
    reproduced twice.  The lazy ramp is load-bearing.
  - Restructured input DMA (wide 512-col weight slices, kt-striped or
    consumption-staged queue orders): bursty arrivals starve the cold PE
    (1-3us holes), HAM stays cold until ~34us: +7us.  The TYPE-per-queue
    baseline layout gives pipelined pairwise arrivals (~650ns/kt) that keep
    the cold matmul stream ~90% busy.
  - reciprocal-before-bounce (in-place custom-DVE recip on the [1,512]
    denominator row, then DMA of that row): NaN on silicon only.
  - gpsimd PSUM->SBUF copies: ILLEGAL (walrus: GPSIMD cannot access PSUM;
    CoreSim does not model this).
  - gpsimd SBUF-only tensor_mul for band masks / normalize: legal but ~7x
    slower than DVE (~15 G elem/s effective): +18us.
  - fp8 (any operand of any matmul): numerically dead for the 2e-2 budget
    (qkv 4.3e-2, o 3.7e-2, proj 3.7e-2; only S fp8 fits at 1.5e-2).
  - O-stream col-tiled head pairing ([128,512] O matmuls, no ones-row):
    the separate denominator ones-matmul costs exactly the streamed
    columns the pairing saves; provably net-zero.
Measured WINS over the 279us baseline (kept in this file):
  - 3-deep exp->O software pipeline: an O matmul issued the moment its exp
    lands pays a pipeline restart (~SBUF access latency); with 2-3 groups
    of slack its pT semaphore is pre-satisfied (O busy 355 -> 292ns/MM).
  - DEFERRED normalize tails: a chunk's recip+mul used to enter the
    strict-FIFO DVE queue waiting ~1.8us on the DRAM-bounce DMA --
    head-of-line blocking the NEXT chunk's band-mask muls, which gate its
    O matmuls (~3.7us PE gap at every group boundary).  Each chunk's
    recip+mul now emit at the START of the next chunk (flush_norm), by
    which time the rb broadcast has landed; proj(qc) emission follows the
    flush of norm(3,qc) so writer-before-reader order is preserved.
  - Final chunk's j1 bounce on the scalar DGE queue (idle after the last
    exp): the two heads' chains gate the last proj group in parallel.
  Together: 284 -> 274us under identical device conditions (279 -> 274 vs
  the cold-device session-start baseline measurement).
Roofline notes: PE column-streaming floor ~190us at 2.4 GHz, ACT exp floor
~163us (1 elem/cycle/lane at 1.2 GHz + 352-cycle per-instruction overhead,
bounded by the 2-bank psS pool), framework preamble ~7us + teardown ~12us.
"""

import os
import numpy as np
import ml_dtypes

B, T, E, H = 4, 2048, 1024, 16
D = E // H            # 64
NCORES = 8
HL = H // 2           # local heads per core
DL = HL * D           # 512 local attention feats
QC = 512              # q-chunk width
NQC = T // QC         # 4
NKT = T // 128        # 16 k-tiles
P = 128

BF16 = ml_dtypes.bfloat16

_graph_cache = {}
LAST_RESULT = None    # BassKernelResults of the most recent run (for test.py)


def _build(causal: bool, with_bias: bool):
    import concourse.bass as bass  # noqa: F401
    import concourse.tile as tile
    from concourse import bacc, mybir
    from concourse.masks import make_upper_triangular

    bf16 = mybir.dt.bfloat16
    f32 = mybir.dt.float32
    Exp = mybir.ActivationFunctionType.Exp

    KIN = 1152 if with_bias else 1024   # qkv contraction (pad bias row to a full tile)
    NKIN = KIN // P

    nc = bacc.Bacc("TRN2", target_bir_lowering=False, debug=False,
                   num_devices=NCORES)
    xT = nc.declare_dram_parameter("xT", [KIN, T], bf16, isOutput=False)
    wqkv = nc.declare_dram_parameter("wqkv", [KIN, 3 * DL], bf16, isOutput=False)
    wproj = nc.declare_dram_parameter("wproj", [DL, E], bf16, isOutput=False)
    if not causal:
        maskT = nc.declare_dram_parameter("maskT", [T, T], bf16, isOutput=False)
    # bf16 output halves the write-out DMA bytes; the host sums the two
    # partials per batch in f32 (error ~5e-3, well under the 2e-2 budget)
    out = nc.declare_dram_parameter("out", [T, E], bf16, isOutput=True)

    with tile.TileContext(nc) as tc, \
         tc.tile_pool(name="persist", bufs=1) as persist:
        # ---- persistent SBUF tensors ----
        xT_sb = persist.tile([P, NKIN, T], bf16, tag="xT_sb", name="xT_sb")
        wq_sb = persist.tile([P, NKIN, 3 * DL], bf16, tag="wq_sb", name="wq_sb")
        wp_sb = persist.tile([P, 4, E], bf16, tag="wp_sb", name="wp_sb")
        qT_sb = persist.tile([P, 4, T], bf16, tag="qT_sb", name="qT_sb")
        kT_sb = persist.tile([P, 4, T], bf16, tag="kT_sb", name="kT_sb")
        vP_sb = persist.tile([P, NKT, HL, D + 1], bf16, tag="vP_sb", name="vP_sb")
        oT_sb = persist.tile([P, 4, T], bf16, tag="oT_sb", name="oT_sb")
        band = persist.tile([P, P], bf16, tag="band", name="band")

        # Input DMA plan (baseline layout -- measured best).  TYPE-per-queue
        # assignment is load-bearing: wq[kt] on sync, wk[kt] on gpsimd and
        # xT-c0[kt] on scalar arrive as perfectly pipelined PAIRS (one kt
        # every ~650ns), so the cold ps_q/ps_k matmul stream runs at ~90%
        # busy and the HAM clock-gate lifts the 1.2 GHz throttle early.
        # kt-striped or stage-batched orders produce bursty arrivals with
        # 1-3us holes that reset the HAM busy window (measured +7us).
        dma_engines = [nc.sync, nc.gpsimd, nc.scalar]
        di = 0
        nq = 3

        def dma_in(out_ap, in_ap):
            nonlocal di
            dma_engines[di % nq].dma_start(out=out_ap, in_=in_ap)
            di += 1

        for kt in range(NKIN):
            dma_in(wq_sb[:, kt, 0:P], wqkv[kt * P:(kt + 1) * P, 0:P])
            dma_in(wq_sb[:, kt, DL:DL + P],
                   wqkv[kt * P:(kt + 1) * P, DL:DL + P])
            dma_in(xT_sb[:, kt, 0:QC], xT[kt * P:(kt + 1) * P, 0:QC])
        nq = 2
        for kt in range(NKIN):
            dma_in(wq_sb[:, kt, 2 * DL:3 * DL],
                   wqkv[kt * P:(kt + 1) * P, 2 * DL:3 * DL])
        for kt in range(NKIN):
            dma_in(xT_sb[:, kt, QC:2 * QC], xT[kt * P:(kt + 1) * P, QC:2 * QC])
        for g in range(1, 4):
            for kt in range(NKIN):
                dma_in(wq_sb[:, kt, g * P:(g + 1) * P],
                       wqkv[kt * P:(kt + 1) * P, g * P:(g + 1) * P])
                dma_in(wq_sb[:, kt, DL + g * P:DL + (g + 1) * P],
                       wqkv[kt * P:(kt + 1) * P, DL + g * P:DL + (g + 1) * P])
        for kt in range(NKIN):
            dma_in(xT_sb[:, kt, 2 * QC:], xT[kt * P:(kt + 1) * P, 2 * QC:])
        for g in range(4):
            dma_in(wp_sb[:, g, :], wproj[g * P:(g + 1) * P, :])
        if causal:
            # band[kp, qf] = 1.0 where kp <= qf else 0  (keep k <= q)
            make_upper_triangular(nc, band[:, :], val=1.0, diag=True)
        nc.vector.memset(vP_sb[:, :, :, D:D + 1], 1.0)
        # preload the ACT exp spline table so the first real exp does not
        # pay the table-switch latency mid-attention
        nc.scalar.activation(out=oT_sb[0:1, 0, 0:1],
                             in_=vP_sb[0:1, 0, 0, D:D + 1], func=Exp)

        with (
            tc.tile_pool(name="psA", bufs=2, space="PSUM") as psA,
            tc.tile_pool(name="psS", bufs=2, space="PSUM") as psS,
            tc.tile_pool(name="psO", bufs=2, space="PSUM") as psO,
            tc.tile_pool(name="sbw", bufs=6) as sbw,
            tc.tile_pool(name="sbm", bufs=6) as sbm,
            tc.tile_pool(name="drp", bufs=2, space="DRAM") as drp,
        ):
            def emit_v(rts, cpy=None):
                # ---- phase 1a: V = x @ Wv  (rows on partitions) ----
                cpy = cpy or nc.vector.tensor_copy
                for rt in rts:
                    ps_v = psA.tile([P, DL], f32, tag="mm512", name="ps_v")
                    for kt in range(NKIN):
                        nc.tensor.matmul(
                            ps_v[:],
                            lhsT=xT_sb[:, kt, rt * P:(rt + 1) * P],
                            rhs=wq_sb[:, kt, 2 * DL:3 * DL],
                            start=(kt == 0), stop=(kt == NKIN - 1))
                    cpy(vP_sb[:, rt, :, 0:D],
                        ps_v[:].rearrange("p (h d) -> p h d", h=HL))

            def emit_qk(g, rcs=None, cpy=None):
                # ---- phase 1b: Q^T, K^T for head-pair g ----
                # (PSUM can only be read by the PE-adjacent engines: DVE and
                # ACT.  GPSIMD cannot access PSUM -- walrus rejects it.)
                cpy = cpy or nc.vector.tensor_copy
                for rc in (range(NQC) if rcs is None else rcs):
                    ps_q = psA.tile([P, QC], f32, tag="mm512", name="ps_q")
                    for kt in range(NKIN):
                        nc.tensor.matmul(
                            ps_q[:],
                            lhsT=wq_sb[:, kt, g * P:(g + 1) * P],
                            rhs=xT_sb[:, kt, rc * QC:(rc + 1) * QC],
                            start=(kt == 0), stop=(kt == NKIN - 1))
                    cpy(qT_sb[:, g, rc * QC:(rc + 1) * QC], ps_q[:])
                    ps_k = psA.tile([P, QC], f32, tag="mm512", name="ps_k")
                    for kt in range(NKIN):
                        nc.tensor.matmul(
                            ps_k[:],
                            lhsT=wq_sb[:, kt, DL + g * P:DL + (g + 1) * P],
                            rhs=xT_sb[:, kt, rc * QC:(rc + 1) * QC],
                            start=(kt == 0), stop=(kt == NKIN - 1))
                    cpy(kT_sb[:, g, rc * QC:(rc + 1) * QC], ps_k[:])

            def emit_proj(rts):
                # ---- phase 3: y_partial = O @ W_proj_shard for row tiles ----
                for rt in rts:
                    for nb in range(2):
                        ps_y = psA.tile([P, 512], f32, tag="mm512", name="ps_y")
                        for g in range(4):
                            nc.tensor.matmul(
                                ps_y[:],
                                lhsT=oT_sb[:, g, rt * P:(rt + 1) * P],
                                rhs=wp_sb[:, g, nb * 512:(nb + 1) * 512],
                                start=(g == 0), stop=(g == 3))
                        y_sb = sbw.tile([P, 512], bf16, tag="y_sb", name="y_sb")
                        nc.vector.tensor_copy(y_sb[:], ps_y[:])
                        (nc.sync if (rt + nb) % 2 else nc.gpsimd).dma_start(
                            out=out[rt * P:(rt + 1) * P, nb * 512:(nb + 1) * 512],
                            in_=y_sb[:])

            # Deferred normalize tails: the recip+mul of a chunk's
            # normalize enter the strict-FIFO DVE queue WAITING on the
            # DRAM-bounce DMA (~1.8us) -- head-of-line blocking the next
            # chunk's band-mask muls, which gate its O matmuls (measured
            # ~3.7us PE gap at every group boundary).  So each chunk's
            # recip+mul are EMITTED at the start of the next chunk, by which
            # time their rb broadcast has long landed.
            pending_norm = []

            def flush_norm(keep=0):
                while len(pending_norm) > keep:
                    pending_norm.pop(0)()

            def emit_attn_qc(g, qc, last=False):
                # ---- phase 2: attention for heads 2g, 2g+1, one q-chunk ----
                # g<3: keep the newest chunk's 2 closures pending one EXTRA
                # chunk (2-chunk deferral) -- at a big->small group boundary
                # the 1-chunk-old recip still waits ~2us on its rb bounce at
                # the DVE FIFO head, blocking the small chunk's band-muls
                # (measured 3.6us PE gap at ~50us).  attn(3) flushes fully at
                # chunk top: proj(prev_qc) emission relies on it.
                flush_norm(2 if g < 3 else 0)
                # O' matmuls are software-pipelined TWO k-groups behind the
                # S^T matmuls: an O matmul that issues right as its exp
                # completes pays a pipeline restart (~SBUF access latency)
                # instead of flowing back-to-back; with 2 groups of slack the
                # pT operand's semaphore is pre-satisfied by the time the PE
                # reaches the O matmul.
                # Each PSUM S^T tile packs BOTH heads' slab for one k-tile as
                # [kpos, j, q]: the two heads' K=64 matmuls land on different
                # PE row halves (tile_position auto-derived from the kT/qT
                # base partition) and different PSUM banks (col 512 is the
                # bank boundary), share one exp-release gate, and are emitted
                # back-to-back -- so each k-tile's S pair runs CONCURRENTLY
                # on the PE array instead of serializing.
                if True:
                    nkt = 4 * (qc + 1) if causal else NKT
                    ps_o = [psO.tile([P, QC], f32, tag="ps_o", name=f"ps_o{j}") for j in range(2)]

                    def emit_o(kt2, pT, ss):
                        # j-inner order alternates the two psO banks so each
                        # matmul's drain overlaps the next one's stream
                        for t2 in range(2):
                            for j in range(2):
                                kt = 2 * kt2 + t2
                                nc.tensor.matmul(
                                    ps_o[j][0:D + 1, ss[t2]:],
                                    lhsT=vP_sb[:, kt, 2 * g + j, :],
                                    rhs=pT[:, t2, j, ss[t2]:],
                                    start=(kt == 0), stop=(kt == nkt - 1))

                    pend = []
                    for kt2 in range(nkt // 2):
                        # live-column start per slab (diagonal tiles are
                        # fully masked below column kt*128 - qc*512)
                        ss = [max(0, (2 * kt2 + t2) * P - qc * QC) if causal else 0
                              for t2 in range(2)]
                        # per-k-tile PSUM tile packs both heads: [kpos, j, q]
                        ps_ss = [psS.tile([P, 2, QC], f32, tag="ps_s",
                                          name=f"ps_s{t2}") for t2 in range(2)]
                        for t2 in range(2):
                            kt = 2 * kt2 + t2
                            for j in range(2):
                                nc.tensor.matmul(
                                    ps_ss[t2][:, j, ss[t2]:],
                                    lhsT=kT_sb[j * D:(j + 1) * D, g, kt * P:(kt + 1) * P],
                                    rhs=qT_sb[j * D:(j + 1) * D, g,
                                              qc * QC + ss[t2]:(qc + 1) * QC],
                                    start=True, stop=True)
                        # pT layout [kpos, t2, j, q]
                        pT = sbw.tile([P, 2, 2, QC], bf16, tag="pT", name="pT")
                        if len(pend) >= 3:
                            emit_o(*pend.pop(0))
                        for t2 in range(2):
                            kt = 2 * kt2 + t2
                            s = ss[t2]
                            nc.scalar.activation(out=pT[:, t2, :, s:],
                                                 in_=ps_ss[t2][:, :, s:],
                                                 func=Exp)
                            if causal:
                                if kt >= 4 * qc:  # diagonal-band k-tile
                                    for j in range(2):
                                        nc.vector.tensor_mul(
                                            pT[:, t2, j, s:s + P],
                                            pT[:, t2, j, s:s + P],
                                            band[:, :])
                            else:
                                msk = sbm.tile([P, QC], bf16, tag="msk", name="msk")
                                nc.sync.dma_start(
                                    out=msk[:],
                                    in_=maskT[kt * P:(kt + 1) * P, qc * QC:(qc + 1) * QC])
                                for j in range(2):
                                    nc.vector.tensor_mul(pT[:, t2, j, :],
                                                         pT[:, t2, j, :], msk[:])
                        pend.append((kt2, pT, ss))
                    for item in pend:
                        emit_o(*item)
                    for j in range(2):
                        # early-release ps_o: copy O + rowsum to SBUF in one
                        # shot, then normalize off-PSUM:  O[d, q] / rowsum[q]
                        oU = sbm.tile([D + 1, QC], f32, tag="oU", name="oU")
                        nc.vector.tensor_copy(oU[:], ps_o[j][0:D + 1, :])
                        rdr = drp.tile([1, QC], f32, tag="rdr", name="rdr")
                        # final chunk only: j1's bounce rides the scalar
                        # queue (HW DGE, idle after the last exp) so the two
                        # heads' chains -- which serially gate the last proj
                        # group -- run in parallel instead of queueing on
                        # sync.  (The earlier NaN here was the in-place row
                        # reciprocal, not the queue choice.)
                        qj = nc.scalar if (last and j == 1) else nc.sync
                        qj.dma_start(out=rdr[:], in_=oU[D:D + 1, :])
                        rb = sbm.tile([D, QC], f32, tag="rb", name="rb")
                        qj.dma_start(out=rb[:],
                                     in_=rdr[:].to_broadcast((D, QC)))

                        def _fin(oU=oU, rb=rb, j=j, g=g, qc=qc):
                            nc.vector.reciprocal_approx_fast(out=rb[:],
                                                             in_=rb[:])
                            nc.vector.tensor_mul(
                                oT_sb[j * D:(j + 1) * D, g,
                                      qc * QC:(qc + 1) * QC],
                                oU[0:D, :], rb[:])
                        pending_norm.append(_fin)
            # emission schedule: the Tile scheduler is a per-engine priority
            # heap (priority = emission order) gated by readiness, so dense
            # matmuls emitted anywhere after a point act as PE filler for the
            # exp-bound attention stream.  The attention phases are ACT-bound
            # (exp deficit ~12us per head-pair); if the PE micro-idles with
            # no ready dense work the HAM clock-gate re-throttles it to
            # 1.2 GHz (baseline lost ~45us to one 75us cold stretch).  So:
            # keep the dense prologue minimal and stagger every remaining
            # dense group across the attention chunks so filler never runs
            # dry: V tail + qk0 tail + qk1 into attn(0), qk2 into attn(1),
            # qk3 (reversed rc, matching attn(3)'s descending qc order) into
            # attn(2), proj per-chunk into attn(3).
            # V and qk0 copies run on the (otherwise idle) scalar engine so
            # the early dense burst leaves no DVE copy backlog to throttle
            # the psA slot turnaround once attention starts.  qk0 rc=0 is
            # first: its inputs land first, so the exp stream starts sooner.
            #
            emit_qk(0, rcs=[0], cpy=nc.scalar.copy)
            emit_v(range(0, 4), cpy=nc.scalar.copy)
            for qc in range(NQC):
                emit_attn_qc(0, qc)
                if qc < NQC - 1:
                    emit_v(range(4 * qc + 4, 4 * qc + 8), cpy=nc.scalar.copy)
                    emit_qk(0, rcs=[qc + 1], cpy=nc.scalar.copy)
                emit_qk(1, rcs=[qc])
            for g in (1, 2):
                for qc in range(NQC):
                    emit_attn_qc(g, qc)
                    emit_qk(g + 1, rcs=[qc if g == 1 else NQC - 1 - qc])
            prev_qc = None
            for qc in range(NQC - 1, -1, -1):
                emit_attn_qc(3, qc, last=(qc == 0))
                if prev_qc is not None:
                    # proj(prev_qc): its norm was flushed at this chunk's top
                    emit_proj(range(4 * prev_qc, 4 * prev_qc + 4))
                prev_qc = qc
            flush_norm()
            emit_proj(range(0, 4))

    nc.compile()
    return nc


def _get_graph(causal: bool, with_bias: bool):
    key = (causal, with_bias)
    if key not in _graph_cache:
        _graph_cache[key] = _build(causal, with_bias)
    return _graph_cache[key]


def make_in_maps(x, mask, W_attn, b_attn, W_proj, b_proj, causal, with_bias):
    """Host-side sharding: per-core input dict (bf16)."""
    in_maps = []
    maskT_bf = None
    if not causal:
        m = np.asarray(mask).reshape(T, T)
        maskT_bf = np.ascontiguousarray(m.T).astype(BF16)
    for c in range(NCORES):
        b, hg = c // 2, c % 2
        lo, hi = hg * DL, (hg + 1) * DL
        Wq = W_attn[:, lo:hi] * np.float32(0.125)
        Wk = W_attn[:, E + lo:E + hi]
        Wv = W_attn[:, 2 * E + lo:2 * E + hi]
        wqkv = np.concatenate([Wq, Wk, Wv], axis=1).astype(np.float32)
        xt = np.ascontiguousarray(x[b].T).astype(np.float32)
        if with_bias:
            brow = np.concatenate([
                b_attn[lo:hi] * np.float32(0.125),
                b_attn[E + lo:E + hi],
                b_attn[2 * E + lo:2 * E + hi]]).astype(np.float32)
            wqkv = np.concatenate(
                [wqkv, brow[None, :], np.zeros((P - 1, 3 * DL), np.float32)], axis=0)
            xt = np.concatenate(
                [xt, np.ones((1, T), np.float32), np.zeros((P - 1, T), np.float32)],
                axis=0)
        im = {
            "xT": np.ascontiguousarray(xt).astype(BF16),
            "wqkv": np.ascontiguousarray(wqkv).astype(BF16),
            "wproj": np.ascontiguousarray(W_proj[lo:hi, :]).astype(BF16),
        }
        if not causal:
            im["maskT"] = maskT_bf
        in_maps.append(im)
    return in_maps


def expected_partial(x, mask, W_attn, b_attn, W_proj, core):
    """Numpy reference for ONE core's partial output (for sim testing)."""
    b, hg = core // 2, core % 2
    lo, hi = hg * DL, (hg + 1) * DL
    q = x[b] @ W_attn[:, lo:hi] + b_attn[lo:hi]
    k = x[b] @ W_attn[:, E + lo:E + hi] + b_attn[E + lo:E + hi]
    v = x[b] @ W_attn[:, 2 * E + lo:2 * E + hi] + b_attn[2 * E + lo:2 * E + hi]
    q = q.reshape(T, HL, D)
    k = k.reshape(T, HL, D)
    v = v.reshape(T, HL, D)
    att = np.einsum('qhd,khd->hqk', q, k) / np.sqrt(D)
    m = np.asarray(mask).reshape(T, T)
    att = np.where(m[None] == 0, np.float32(-1e20), att)
    att = att - att.max(axis=-1, keepdims=True)
    att = np.exp(att)
    att = att / att.sum(axis=-1, keepdims=True)
    o = np.einsum('hqk,khd->qhd', att, v).reshape(T, DL)
    return o @ W_proj[lo:hi, :]


def kernel(x, mask, W_attn, b_attn, W_proj, b_proj):
    global LAST_RESULT
    from concourse.bass_utils import run_bass_kernel_spmd

    x = np.asarray(x, dtype=np.float32)
    W_attn = np.asarray(W_attn, dtype=np.float32)
    b_attn = np.asarray(b_attn, dtype=np.float32)
    W_proj = np.asarray(W_proj, dtype=np.float32)
    b_proj = np.asarray(b_proj, dtype=np.float32)

    mask2d = np.asarray(mask).reshape(T, T)
    causal = bool(np.array_equal(mask2d != 0, np.tril(np.ones((T, T), bool))))
    if not causal and not (mask2d != 0).any(axis=1).all():
        # A fully-masked query row: reference softmax degenerates to uniform
        # attention; not representable in the 0/1-multiply fast path.  This
        # cannot occur for the causal mask; fall back to exact host math.
        y = np.stack([
            sum(expected_partial(x, mask, W_attn, b_attn, W_proj, 2 * b + hg)
                for hg in range(2))
            for b in range(B)]).astype(np.float32)
        return y + b_proj
    with_bias = bool(np.any(b_attn))

    nc = _get_graph(causal, with_bias)
    in_maps = make_in_maps(x, mask, W_attn, b_attn, W_proj, b_proj,
                           causal, with_bias)
    trace = bool(int(os.environ.get("CK_TRACE", "0")))
    res = run_bass_kernel_spmd(nc, in_maps, core_ids=list(range(NCORES)),
                               trace=trace)
    LAST_RESULT = res
    y = np.empty((B, T, E), np.float32)
    for b in range(B):
        y[b] = res.results[2 * b]["out"].astype(np.float32) \
             + res.results[2 * b + 1]["out"].astype(np.float32)
    return y + b_proj



# revision 55
# speedup vs baseline: 1.0024x; 1.0024x over previous
"""Trainium2 (8 NeuronCores) kernel for a GPT-2 style causal attention block.

Reference math (per batch b):
    qkv = x @ W_attn + b_attn            # [T, 3E]
    q,k,v split -> heads H=16, D=64
    att = softmax(mask(q k^T / sqrt(D))) # causal mask
    y   = (att @ v) @ W_proj + b_proj    # [T, E]

Sharding (8 cores, no collectives):
    core c = (batch b = c//2, head-group hg = c%2 of 8 heads).
    Each core computes a PARTIAL y[b] = O_local @ W_proj[rows of its heads].
    Host sums the two partials per batch and adds b_proj (exact, commutes).

Device kernel per core (all bf16 matmuls, fp32 PSUM accumulate):
    phase 1: Q^T, K^T (feats on partitions) and V (rows on partitions) via
             matmuls from host-fed x^T and W shards.  1/sqrt(D) is folded
             into the Q columns of W on the host (exact: /8 is a pow2).
    phase 2: per (head-pair, q-chunk of 512): S^T tiles [128 k, 512 q] on
             PE, exp on ACT (no max-subtraction needed: scores are O(1) by
             construction).  The two heads' K=64 S^T matmuls are packed into
             one PSUM tile [kpos, j, q] and emitted back-to-back: they land
             on different PE row halves (tile_position auto-derived from the
             kT/qT base partition) and different PSUM banks, so each k-tile's
             S pair runs CONCURRENTLY on the array (~2x on the S stream).
             Causal structure: k-tiles above the diagonal are skipped,
             matmuls/exp on diagonal tiles are trimmed to live columns, and
             the 128-wide mixed band is masked by multiplying with a 128x128
             triangular tile.  O'^T accumulates with a V' that has a
             ones-column appended -> row 64 of O' is the softmax
             denominator.  Normalization happens off-PSUM: one [65,512] copy,
             DRAM-bounce broadcast of the denominator row, fast reciprocal,
             multiply into O^T.
    phase 3: y_partial = O @ W_proj_shard, PSUM -> SBUF -> DRAM (bf16;
             the host sums the two partials per batch in f32).

Schedule (the real perf lever on HW): the Tile scheduler is a per-engine
priority heap (priority = emission order) gated by readiness.  The
attention phases are ACT(exp)-paced, and any PE micro-idle risks the HAM
clock gate re-throttling the PE to 1.2 GHz (the original schedule lost
~45us to one 75us cold stretch).  So the dense matmul work (V, QK, proj)
is staggered across the attention chunks as filler: minimal prologue
(qk0 rc0 first -- its inputs land first -- then V rt0-3), V tail + qk0
tail + qk1 into attn(0), qk2 into attn(1), qk3 (reversed rc to match
attn(3)'s descending qc order) into attn(2), proj per-chunk into
attn(3).  Early-phase PSUM->SBUF copies run on the (exp-idle) scalar
engine so the DVE has no copy backlog that would throttle the psA slot
turnaround when attention starts.

Measured on HW and rejected (CoreSim passes, silicon disagrees): K=1 PE
broadcast matmuls (garbage), gpsimd partition_broadcast (NaN), a merged
4-bank exp tile (PSUM single-buffering stalls), late qk3 emission (NaN),
finer input-DMA slicing (DGE trigger-bound).

Also measured and rejected in a later session (each vs the 279us baseline):
  - PE pre-warm dummy matmuls during the DMA ramp: lifts the HAM 1.2 GHz
    cold throttle ~5us earlier, but the denser early profile (x8 cores)
    trips the chip-level P0 power limiter and the whole body then runs at
    2.0 instead of 2.4 GHz (dense MM spacing 259ns vs 216ns): 285 -> 332us,
    reproduced twice.  The lazy ramp is load-bearing.
  - Restructured input DMA (wide 512-col weight slices, kt-striped or
    consumption-staged queue orders): bursty arrivals starve the cold PE
    (1-3us holes), HAM stays cold until ~34us: +7us.  The TYPE-per-queue
    baseline layout gives pipelined pairwise arrivals (~650ns/kt) that keep
    the cold matmul stream ~90% busy.
  - reciprocal-before-bounce (in-place custom-DVE recip on the [1,512]
    denominator row, then DMA of that row): NaN on silicon only.
  - gpsimd PSUM->SBUF copies: ILLEGAL (walrus: GPSIMD cannot access PSUM;
    CoreSim does not model this).
  - gpsimd SBUF-only tensor_mul for band masks / normalize: legal but ~7x
    slower than DVE (~15 G elem/s effective): +18us.
  - fp8 (any operand of any matmul): numerically dead for the 2e-2 budget
    (qkv 4.3e-2, o 3.7e-2, proj 3.7e-2; only S fp8 fits at 1.5e-2).
  - O-stream col-tiled head pairing ([128,512] O matmuls, no ones-row):
    the separate denominator ones-matmul costs exactly the streamed
    columns the pairing saves; provably net-zero.
Measured WINS over the 279us baseline (kept in this file):
  - 3-deep exp->O software pipeline: an O matmul issued the moment its exp
    lands pays a pipeline restart (~SBUF access latency); with 2-3 groups
    of slack its pT semaphore is pre-satisfied (O busy 355 -> 292ns/MM).
  - DEFERRED normalize tails: a chunk's recip+mul used to enter the
    strict-FIFO DVE queue waiting ~1.8us on the DRAM-bounce DMA --
    head-of-line blocking the NEXT chunk's band-mask muls, which gate its
    O matmuls (~3.7us PE gap at every group boundary).  Each chunk's
    recip+mul now emit at the START of the next chunk (flush_norm), by
    which time the rb broadcast has landed; proj(qc) emission follows the
    flush of norm(3,qc) so writer-before-reader order is preserved.
  - Final chunk's j1 bounce on the scalar DGE queue (idle after the last
    exp): the two heads' chains gate the last proj group in parallel.
  Together: 284 -> 274us under identical device conditions (279 -> 274 vs
  the cold-device session-start baseline measurement).
Roofline notes: PE column-streaming floor ~190us at 2.4 GHz, ACT exp floor
~163us (1 elem/cycle/lane at 1.2 GHz + 352-cycle per-instruction overhead,
bounded by the 2-bank psS pool), framework preamble ~7us + teardown ~12us.
"""

import os
import numpy as np
import ml_dtypes

B, T, E, H = 4, 2048, 1024, 16
D = E // H            # 64
NCORES = 8
HL = H // 2           # local heads per core
DL = HL * D           # 512 local attention feats
QC = 512              # q-chunk width
NQC = T // QC         # 4
NKT = T // 128        # 16 k-tiles
P = 128

BF16 = ml_dtypes.bfloat16

_graph_cache = {}
LAST_RESULT = None    # BassKernelResults of the most recent run (for test.py)


def _build(causal: bool, with_bias: bool):
    import concourse.bass as bass  # noqa: F401
    import concourse.tile as tile
    from concourse import bacc, mybir
    from concourse.masks import make_upper_triangular

    bf16 = mybir.dt.bfloat16
    f32 = mybir.dt.float32
    Exp = mybir.ActivationFunctionType.Exp

    KIN = 1152 if with_bias else 1024   # qkv contraction (pad bias row to a full tile)
    NKIN = KIN // P

    nc = bacc.Bacc("TRN2", target_bir_lowering=False, debug=False,
                   num_devices=NCORES)
    xT = nc.declare_dram_parameter("xT", [KIN, T], bf16, isOutput=False)
    wqkv = nc.declare_dram_parameter("wqkv", [KIN, 3 * DL], bf16, isOutput=False)
    wproj = nc.declare_dram_parameter("wproj", [DL, E], bf16, isOutput=False)
    if not causal:
        maskT = nc.declare_dram_parameter("maskT", [T, T], bf16, isOutput=False)
    # bf16 output halves the write-out DMA bytes; the host sums the two
    # partials per batch in f32 (error ~5e-3, well under the 2e-2 budget)
    out = nc.declare_dram_parameter("out", [T, E], bf16, isOutput=True)

    with tile.TileContext(nc) as tc, \
         tc.tile_pool(name="persist", bufs=1) as persist:
        # ---- persistent SBUF tensors ----
        xT_sb = persist.tile([P, NKIN, T], bf16, tag="xT_sb", name="xT_sb")
        wq_sb = persist.tile([P, NKIN, 3 * DL], bf16, tag="wq_sb", name="wq_sb")
        wp_sb = persist.tile([P, 4, E], bf16, tag="wp_sb", name="wp_sb")
        qT_sb = persist.tile([P, 4, T], bf16, tag="qT_sb", name="qT_sb")
        kT_sb = persist.tile([P, 4, T], bf16, tag="kT_sb", name="kT_sb")
        vP_sb = persist.tile([P, NKT, HL, D + 1], bf16, tag="vP_sb", name="vP_sb")
        oT_sb = persist.tile([P, 4, T], bf16, tag="oT_sb", name="oT_sb")
        band = persist.tile([P, P], bf16, tag="band", name="band")

        # Input DMA plan (baseline layout -- measured best).  TYPE-per-queue
        # assignment is load-bearing: wq[kt] on sync, wk[kt] on gpsimd and
        # xT-c0[kt] on scalar arrive as perfectly pipelined PAIRS (one kt
        # every ~650ns), so the cold ps_q/ps_k matmul stream runs at ~90%
        # busy and the HAM clock-gate lifts the 1.2 GHz throttle early.
        # kt-striped or stage-batched orders produce bursty arrivals with
        # 1-3us holes that reset the HAM busy window (measured +7us).
        dma_engines = [nc.sync, nc.gpsimd, nc.scalar]
        di = 0
        nq = 3

        def dma_in(out_ap, in_ap):
            nonlocal di
            dma_engines[di % nq].dma_start(out=out_ap, in_=in_ap)
            di += 1

        for kt in range(NKIN):
            dma_in(wq_sb[:, kt, 0:P], wqkv[kt * P:(kt + 1) * P, 0:P])
            dma_in(wq_sb[:, kt, DL:DL + P],
                   wqkv[kt * P:(kt + 1) * P, DL:DL + P])
            dma_in(xT_sb[:, kt, 0:QC], xT[kt * P:(kt + 1) * P, 0:QC])
        nq = 2
        for kt in range(NKIN):
            dma_in(wq_sb[:, kt, 2 * DL:3 * DL],
                   wqkv[kt * P:(kt + 1) * P, 2 * DL:3 * DL])
        for kt in range(NKIN):
            dma_in(xT_sb[:, kt, QC:2 * QC], xT[kt * P:(kt + 1) * P, QC:2 * QC])
        for g in range(1, 4):
            for kt in range(NKIN):
                dma_in(wq_sb[:, kt, g * P:(g + 1) * P],
                       wqkv[kt * P:(kt + 1) * P, g * P:(g + 1) * P])
                dma_in(wq_sb[:, kt, DL + g * P:DL + (g + 1) * P],
                       wqkv[kt * P:(kt + 1) * P, DL + g * P:DL + (g + 1) * P])
        for kt in range(NKIN):
            dma_in(xT_sb[:, kt, 2 * QC:], xT[kt * P:(kt + 1) * P, 2 * QC:])
        for g in range(4):
            dma_in(wp_sb[:, g, :], wproj[g * P:(g + 1) * P, :])
        if causal:
            # band[kp, qf] = 1.0 where kp <= qf else 0  (keep k <= q)
            make_upper_triangular(nc, band[:, :], val=1.0, diag=True)
        nc.vector.memset(vP_sb[:, :, :, D:D + 1], 1.0)
        # preload the ACT exp spline table so the first real exp does not
        # pay the table-switch latency mid-attention
        nc.scalar.activation(out=oT_sb[0:1, 0, 0:1],
                             in_=vP_sb[0:1, 0, 0, D:D + 1], func=Exp)

        with (
            tc.tile_pool(name="psA", bufs=2, space="PSUM") as psA,
            tc.tile_pool(name="psS", bufs=2, space="PSUM") as psS,
            tc.tile_pool(name="psO", bufs=2, space="PSUM") as psO,
            tc.tile_pool(name="sbw", bufs=6) as sbw,
            tc.tile_pool(name="sbm", bufs=4) as sbm,
            tc.tile_pool(name="drp", bufs=2, space="DRAM") as drp,
        ):
            def emit_v(rts, cpy=None):
                # ---- phase 1a: V = x @ Wv  (rows on partitions) ----
                cpy = cpy or nc.vector.tensor_copy
                for rt in rts:
                    ps_v = psA.tile([P, DL], f32, tag="mm512", name="ps_v")
                    for kt in range(NKIN):
                        nc.tensor.matmul(
                            ps_v[:],
                            lhsT=xT_sb[:, kt, rt * P:(rt + 1) * P],
                            rhs=wq_sb[:, kt, 2 * DL:3 * DL],
                            start=(kt == 0), stop=(kt == NKIN - 1))
                    cpy(vP_sb[:, rt, :, 0:D],
                        ps_v[:].rearrange("p (h d) -> p h d", h=HL))

            def emit_qk(g, rcs=None, cpy=None):
                # ---- phase 1b: Q^T, K^T for head-pair g ----
                # (PSUM can only be read by the PE-adjacent engines: DVE and
                # ACT.  GPSIMD cannot access PSUM -- walrus rejects it.)
                cpy = cpy or nc.vector.tensor_copy
                for rc in (range(NQC) if rcs is None else rcs):
                    ps_q = psA.tile([P, QC], f32, tag="mm512", name="ps_q")
                    for kt in range(NKIN):
                        nc.tensor.matmul(
                            ps_q[:],
                            lhsT=wq_sb[:, kt, g * P:(g + 1) * P],
                            rhs=xT_sb[:, kt, rc * QC:(rc + 1) * QC],
                            start=(kt == 0), stop=(kt == NKIN - 1))
                    cpy(qT_sb[:, g, rc * QC:(rc + 1) * QC], ps_q[:])
                    ps_k = psA.tile([P, QC], f32, tag="mm512", name="ps_k")
                    for kt in range(NKIN):
                        nc.tensor.matmul(
                            ps_k[:],
                            lhsT=wq_sb[:, kt, DL + g * P:DL + (g + 1) * P],
                            rhs=xT_sb[:, kt, rc * QC:(rc + 1) * QC],
                            start=(kt == 0), stop=(kt == NKIN - 1))
                    cpy(kT_sb[:, g, rc * QC:(rc + 1) * QC], ps_k[:])

            def emit_proj(rts):
                # ---- phase 3: y_partial = O @ W_proj_shard for row tiles ----
                for rt in rts:
                    for nb in range(2):
                        ps_y = psA.tile([P, 512], f32, tag="mm512", name="ps_y")
                        for g in range(4):
                            nc.tensor.matmul(
                                ps_y[:],
                                lhsT=oT_sb[:, g, rt * P:(rt + 1) * P],
                                rhs=wp_sb[:, g, nb * 512:(nb + 1) * 512],
                                start=(g == 0), stop=(g == 3))
                        y_sb = sbw.tile([P, 512], bf16, tag="y_sb", name="y_sb")
                        nc.vector.tensor_copy(y_sb[:], ps_y[:])
                        (nc.sync if (rt + nb) % 2 else nc.gpsimd).dma_start(
                            out=out[rt * P:(rt + 1) * P, nb * 512:(nb + 1) * 512],
                            in_=y_sb[:])

            # Deferred normalize tails: the recip+mul of a chunk's
            # normalize enter the strict-FIFO DVE queue WAITING on the
            # DRAM-bounce DMA (~1.8us) -- head-of-line blocking the next
            # chunk's band-mask muls, which gate its O matmuls (measured
            # ~3.7us PE gap at every group boundary).  So each chunk's
            # recip+mul are EMITTED at the start of the next chunk, by which
            # time their rb broadcast has long landed.
            pending_norm = []

            def flush_norm():
                for fn in pending_norm:
                    fn()
                pending_norm.clear()

            def emit_attn_qc(g, qc, last=False):
                # ---- phase 2: attention for heads 2g, 2g+1, one q-chunk ----
                flush_norm()
                # O' matmuls are software-pipelined TWO k-groups behind the
                # S^T matmuls: an O matmul that issues right as its exp
                # completes pays a pipeline restart (~SBUF access latency)
                # instead of flowing back-to-back; with 2 groups of slack the
                # pT operand's semaphore is pre-satisfied by the time the PE
                # reaches the O matmul.
                # Each PSUM S^T tile packs BOTH heads' slab for one k-tile as
                # [kpos, j, q]: the two heads' K=64 matmuls land on different
                # PE row halves (tile_position auto-derived from the kT/qT
                # base partition) and different PSUM banks (col 512 is the
                # bank boundary), share one exp-release gate, and are emitted
                # back-to-back -- so each k-tile's S pair runs CONCURRENTLY
                # on the PE array instead of serializing.
                if True:
                    nkt = 4 * (qc + 1) if causal else NKT
                    ps_o = [psO.tile([P, QC], f32, tag="ps_o", name=f"ps_o{j}") for j in range(2)]

                    def emit_o(kt2, pT, ss):
                        # j-inner order alternates the two psO banks so each
                        # matmul's drain overlaps the next one's stream
                        for t2 in range(2):
                            for j in range(2):
                                kt = 2 * kt2 + t2
                                nc.tensor.matmul(
                                    ps_o[j][0:D + 1, ss[t2]:],
                                    lhsT=vP_sb[:, kt, 2 * g + j, :],
                                    rhs=pT[:, t2, j, ss[t2]:],
                                    start=(kt == 0), stop=(kt == nkt - 1))

                    pend = []
                    for kt2 in range(nkt // 2):
                        # live-column start per slab (diagonal tiles are
                        # fully masked below column kt*128 - qc*512)
                        ss = [max(0, (2 * kt2 + t2) * P - qc * QC) if causal else 0
                              for t2 in range(2)]
                        # per-k-tile PSUM tile packs both heads: [kpos, j, q]
                        ps_ss = [psS.tile([P, 2, QC], f32, tag="ps_s",
                                          name=f"ps_s{t2}") for t2 in range(2)]
                        for t2 in range(2):
                            kt = 2 * kt2 + t2
                            for j in range(2):
                                nc.tensor.matmul(
                                    ps_ss[t2][:, j, ss[t2]:],
                                    lhsT=kT_sb[j * D:(j + 1) * D, g, kt * P:(kt + 1) * P],
                                    rhs=qT_sb[j * D:(j + 1) * D, g,
                                              qc * QC + ss[t2]:(qc + 1) * QC],
                                    start=True, stop=True)
                        # pT layout [kpos, t2, j, q]
                        pT = sbw.tile([P, 2, 2, QC], bf16, tag="pT", name="pT")
                        if len(pend) >= 3:
                            emit_o(*pend.pop(0))
                        for t2 in range(2):
                            kt = 2 * kt2 + t2
                            s = ss[t2]
                            nc.scalar.activation(out=pT[:, t2, :, s:],
                                                 in_=ps_ss[t2][:, :, s:],
                                                 func=Exp)
                            if causal:
                                if kt >= 4 * qc:  # diagonal-band k-tile
                                    for j in range(2):
                                        nc.vector.tensor_mul(
                                            pT[:, t2, j, s:s + P],
                                            pT[:, t2, j, s:s + P],
                                            band[:, :])
                            else:
                                msk = sbm.tile([P, QC], bf16, tag="msk", name="msk")
                                nc.sync.dma_start(
                                    out=msk[:],
                                    in_=maskT[kt * P:(kt + 1) * P, qc * QC:(qc + 1) * QC])
                                for j in range(2):
                                    nc.vector.tensor_mul(pT[:, t2, j, :],
                                                         pT[:, t2, j, :], msk[:])
                        pend.append((kt2, pT, ss))
                    for item in pend:
                        emit_o(*item)
                    for j in range(2):
                        # early-release ps_o: copy O + rowsum to SBUF in one
                        # shot, then normalize off-PSUM:  O[d, q] / rowsum[q]
                        oU = sbm.tile([D + 1, QC], f32, tag="oU", name="oU")
                        nc.vector.tensor_copy(oU[:], ps_o[j][0:D + 1, :])
                        rdr = drp.tile([1, QC], f32, tag="rdr", name="rdr")
                        # final chunk only: j1's bounce rides the scalar
                        # queue (HW DGE, idle after the last exp) so the two
                        # heads' chains -- which serially gate the last proj
                        # group -- run in parallel instead of queueing on
                        # sync.  (The earlier NaN here was the in-place row
                        # reciprocal, not the queue choice.)
                        qj = nc.scalar if (last and j == 1) else nc.sync
                        qj.dma_start(out=rdr[:], in_=oU[D:D + 1, :])
                        rb = sbm.tile([D, QC], f32, tag="rb", name="rb")
                        qj.dma_start(out=rb[:],
                                     in_=rdr[:].to_broadcast((D, QC)))

                        def _fin(oU=oU, rb=rb, j=j, g=g, qc=qc):
                            nc.vector.reciprocal_approx_fast(out=rb[:],
                                                             in_=rb[:])
                            nc.vector.tensor_mul(
                                oT_sb[j * D:(j + 1) * D, g,
                                      qc * QC:(qc + 1) * QC],
                                oU[0:D, :], rb[:])
                        pending_norm.append(_fin)
            # emission schedule: the Tile scheduler is a per-engine priority
            # heap (priority = emission order) gated by readiness, so dense
            # matmuls emitted anywhere after a point act as PE filler for the
            # exp-bound attention stream.  The attention phases are ACT-bound
            # (exp deficit ~12us per head-pair); if the PE micro-idles with
            # no ready dense work the HAM clock-gate re-throttles it to
            # 1.2 GHz (baseline lost ~45us to one 75us cold stretch).  So:
            # keep the dense prologue minimal and stagger every remaining
            # dense group across the attention chunks so filler never runs
            # dry: V tail + qk0 tail + qk1 into attn(0), qk2 into attn(1),
            # qk3 (reversed rc, matching attn(3)'s descending qc order) into
            # attn(2), proj per-chunk into attn(3).
            # V and qk0 copies run on the (otherwise idle) scalar engine so
            # the early dense burst leaves no DVE copy backlog to throttle
            # the psA slot turnaround once attention starts.  qk0 rc=0 is
            # first: its inputs land first, so the exp stream starts sooner.
            #
            emit_qk(0, rcs=[0], cpy=nc.scalar.copy)
            emit_v(range(0, 4), cpy=nc.scalar.copy)
            for qc in range(NQC):
                emit_attn_qc(0, qc)
                if qc < NQC - 1:
                    emit_v(range(4 * qc + 4, 4 * qc + 8), cpy=nc.scalar.copy)
                    emit_qk(0, rcs=[qc + 1], cpy=nc.scalar.copy)
                emit_qk(1, rcs=[qc])
            for g in (1, 2):
                for qc in range(NQC):
                    emit_attn_qc(g, qc)
                    emit_qk(g + 1, rcs=[qc if g == 1 else NQC - 1 - qc])
            prev_qc = None
            for qc in range(NQC - 1, -1, -1):
                emit_attn_qc(3, qc, last=(qc == 0))
                if prev_qc is not None:
                    # proj(prev_qc): its norm was flushed at this chunk's top
                    emit_proj(range(4 * prev_qc, 4 * prev_qc + 4))
                prev_qc = qc
            flush_norm()
            emit_proj(range(0, 4))

    nc.compile()
    return nc


def _get_graph(causal: bool, with_bias: bool):
    key = (causal, with_bias)
    if key not in _graph_cache:
        _graph_cache[key] = _build(causal, with_bias)
    return _graph_cache[key]


def make_in_maps(x, mask, W_attn, b_attn, W_proj, b_proj, causal, with_bias):
    """Host-side sharding: per-core input dict (bf16)."""
    in_maps = []
    maskT_bf = None
    if not causal:
        m = np.asarray(mask).reshape(T, T)
        maskT_bf = np.ascontiguousarray(m.T).astype(BF16)
    for c in range(NCORES):
        b, hg = c // 2, c % 2
        lo, hi = hg * DL, (hg + 1) * DL
        Wq = W_attn[:, lo:hi] * np.float32(0.125)
        Wk = W_attn[:, E + lo:E + hi]
        Wv = W_attn[:, 2 * E + lo:2 * E + hi]
        wqkv = np.concatenate([Wq, Wk, Wv], axis=1).astype(np.float32)
        xt = np.ascontiguousarray(x[b].T).astype(np.float32)
        if with_bias:
            brow = np.concatenate([
                b_attn[lo:hi] * np.float32(0.125),
                b_attn[E + lo:E + hi],
                b_attn[2 * E + lo:2 * E + hi]]).astype(np.float32)
            wqkv = np.concatenate(
                [wqkv, brow[None, :], np.zeros((P - 1, 3 * DL), np.float32)], axis=0)
            xt = np.concatenate(
                [xt, np.ones((1, T), np.float32), np.zeros((P - 1, T), np.float32)],
                axis=0)
        im = {
            "xT": np.ascontiguousarray(xt).astype(BF16),
            "wqkv": np.ascontiguousarray(wqkv).astype(BF16),
            "wproj": np.ascontiguousarray(W_proj[lo:hi, :]).astype(BF16),
        }
        if not causal:
            im["maskT"] = maskT_bf
        in_maps.append(im)
    return in_maps


def expected_partial(x, mask, W_attn, b_attn, W_proj, core):
    """Numpy reference for ONE core's partial output (for sim testing)."""
    b, hg = core // 2, core % 2
    lo, hi = hg * DL, (hg + 1) * DL
    q = x[b] @ W_attn[:, lo:hi] + b_attn[lo:hi]
    k = x[b] @ W_attn[:, E + lo:E + hi] + b_attn[E + lo:E + hi]
    v = x[b] @ W_attn[:, 2 * E + lo:2 * E + hi] + b_attn[2 * E + lo:2 * E + hi]
    q = q.reshape(T, HL, D)
    k = k.reshape(T, HL, D)
    v = v.reshape(T, HL, D)
    att = np.einsum('qhd,khd->hqk', q, k) / np.sqrt(D)
    m = np.asarray(mask).reshape(T, T)
    att = np.where(m[None] == 0, np.float32(-1e20), att)
    att = att - att.max(axis=-1, keepdims=True)
    att = np.exp(att)
    att = att / att.sum(axis=-1, keepdims=True)
    o = np.einsum('hqk,khd->qhd', att, v).reshape(T, DL)
    return o @ W_proj[lo:hi, :]


def kernel(x, mask, W_attn, b_attn, W_proj, b_proj):
    global LAST_RESULT
    from concourse.bass_utils import run_bass_kernel_spmd

    x = np.asarray(x, dtype=np.float32)
    W_attn = np.asarray(W_attn, dtype=np.float32)
    b_attn = np.asarray(b_attn, dtype=np.float32)
    W_proj = np.asarray(W_proj, dtype=np.float32)
    b_proj = np.asarray(b_proj, dtype=np.float32)

    mask2d = np.asarray(mask).reshape(T, T)
    causal = bool(np.array_equal(mask2d != 0, np.tril(np.ones((T, T), bool))))
    if not causal and not (mask2d != 0).any(axis=1).all():
        # A fully-masked query row: reference softmax degenerates to uniform
        # attention; not representable in the 0/1-multiply fast path.  This
        # cannot occur for the causal mask; fall back to exact host math.
        y = np.stack([
            sum(expected_partial(x, mask, W_attn, b_attn, W_proj, 2 * b + hg)
                for hg in range(2))
            for b in range(B)]).astype(np.float32)
        return y + b_proj
    with_bias = bool(np.any(b_attn))

    nc = _get_graph(causal, with_bias)
    in_maps = make_in_maps(x, mask, W_attn, b_attn, W_proj, b_proj,
                           causal, with_bias)
    trace = bool(int(os.environ.get("CK_TRACE", "0")))
    res = run_bass_kernel_spmd(nc, in_maps, core_ids=list(range(NCORES)),
                               trace=trace)
    LAST_RESULT = res
    y = np.empty((B, T, E), np.float32)
    for b in range(B):
        y[b] = res.results[2 * b]["out"].astype(np.float32) \
             + res.results[2 * b + 1]["out"].astype(np.float32)
    return y + b_proj

